# revision 19
# baseline (speedup 1.0000x reference)
import os
import sys

for _p in ("/opt/trn_rl_repo", "/root/.axon_site/_ro/trn_rl_repo"):
    if os.path.isdir(_p) and _p not in sys.path:
        sys.path.insert(0, _p)

import numpy as np

import concourse.bass as bass
import concourse.mybir as mybir
from concourse.tile import TileContext
from concourse import bass_utils
from concourse import bacc

F16 = mybir.dt.float16
F32 = mybir.dt.float32
I16 = mybir.dt.int16
I32 = mybir.dt.int32
AF = mybir.ActivationFunctionType
OP = mybir.AluOpType

N_CORES = 8
BATCH = 65536
C = 4                  # classes
T = 120                # time steps
PB = BATCH // N_CORES  # batch per core = 8192
G = 32                 # groups (partitions = 4 classes x 32 groups)
FB = PB // G           # free-dim batch per partition = 256
HB = FB // 2           # half-width per stream = 128
K = 16                 # epoch length (0.8^-u stays fp16-safe)
OCT = 8                # octet length for bookkeeping
NOCT = T // OCT        # 15 octets
CH = 8                 # dpn steps per DMA chunk
DT_MS = 10.0
THR = 2.5              # threshold in S=5*acc units
EPS5 = 5e-9
DEC = 0.8


def _softplus(x):
    return np.logaddexp(0.0, x.astype(np.float64)).astype(np.float32)


def _build(nc):
    dpn_d = nc.dram_tensor("dpn", [T // CH, 128, CH * FB], F16, kind="ExternalInput")
    w_d = nc.dram_tensor("wmats", [128, 17 * 128], F16, kind="ExternalInput")
    out_d = nc.dram_tensor("out", [128, FB], F32, kind="ExternalOutput")

    with TileContext(nc) as tc:
        with (
            tc.tile_pool(name="persist", bufs=1) as persist,
            tc.tile_pool(name="dpnp", bufs=3) as dpnp,
            tc.tile_pool(name="work", bufs=2) as work,
            tc.tile_pool(name="psum", bufs=1, space="PSUM") as psump,
        ):
            # --- stationary weights: W_j (j=0..14), W_m1=A, Wcar=0.8*A ---
            wall = persist.tile([128, 17 * 128], F16, name="wall")
            nc.sync.dma_start(wall[:], w_d[:])
            Wj = [wall[:, j * 128:(j + 1) * 128] for j in range(15)]
            Wm1 = wall[:, 15 * 128:16 * 128]
            Wcar = wall[:, 16 * 128:17 * 128]
            ident = persist.tile([128, 128], F16, name="ident")
            rowi = persist.tile([128, 128], I32, name="rowi")
            nc.gpsimd.iota(rowi[:], [[1, 128]], base=0, channel_multiplier=0)
            coli = persist.tile([128, 1], I32, name="coli")
            nc.gpsimd.iota(coli[:], [[0, 1]], base=0, channel_multiplier=1)
            row = persist.tile([128, 128], F32, name="row")
            nc.vector.tensor_copy(row[:], rowi[:])
            col = persist.tile([128, 1], F32, name="col")
            nc.vector.tensor_copy(col[:], coli[:])
            nc.vector.tensor_scalar(ident[:], row[:], col[:], None, OP.is_equal)

            # --- state rings (slot t at ring position t%16; each slot holds
            # stream A in cols [0:HB) and stream B in cols [HB:FB)) ---
            spr = persist.tile([128, 16 * FB], F16, name="spr")   # sp ring
            Sr = persist.tile([128, 16 * FB], F16, name="Sr")     # S ring
            nc.vector.memset(Sr[:, 15 * FB:16 * FB], 0.0)         # S_{-1} = 0
            Fo = persist.tile([128, FB], F16, name="Fo")
            nc.vector.memset(Fo[:], 0.0)
            cnt = persist.tile([128, FB], I32, name="cnt")
            nc.vector.memset(cnt[:], 1)                           # octet-0 latch
            capS = persist.tile([128, FB], F16, name="capS")
            capSP = persist.tile([128, OCT * FB], F16, name="capSP")
            nc.vector.memset(capS[:], 0.0)
            nc.vector.memset(capSP[:], 0.0)
            nfo = [persist.tile([128, FB], I32, name=f"nfo{i}") for i in range(2)]
            nfow = [persist.tile([128, OCT * FB], I16, name=f"nfow{i}") for i in range(2)]
            nc.vector.memset(nfo[0][:], 1)
            nc.vector.memset(nfow[0][:], 1)
            L1t = persist.tile([128, 4 * FB], F16, name="L1t")
            L2t = persist.tile([128, 2 * FB], F16, name="L2t")
            L3t = persist.tile([128, FB], F16, name="L3t")

            # --- PSUM accumulators ---
            P0 = psump.tile([128, FB], F32, name="P0")
            P1 = psump.tile([128, FB], F32, name="P1")
            E0 = psump.tile([128, FB], F32, name="E0")
            E1 = psump.tile([128, FB], F32, name="E1")
            Ps = [P0, P1]

            dpn_t = [None] * (T // CH)
            dt0 = dpnp.tile([128, CH * FB], F16, tag="dpn", name="dp0")
            nc.sync.dma_start(dt0[:], dpn_d[0])
            dpn_t[0] = dt0
            nc.tensor.matmul(P0[:], ident[:], dt0[:, 0:FB], start=True, stop=False)
            spv = None  # sp ring slot AP of previous step

            for t in range(T):
                u = t % K
                e = t // K
                P = Ps[e % 2]
                Pn = Ps[(e + 1) % 2]
                Etile = E0 if (t % 2 == 0) else E1

                # --- PE: carry for next epoch (start=True on Pn) first ---
                if u == K - 1 and t < T - 1:
                    sprev2 = Sr[:, ((t - 1) % 16) * FB:((t - 1) % 16 + 1) * FB]
                    nc.tensor.matmul(Pn[:], Wcar, sprev2, start=True, stop=False)

                # --- PE: z of previous sp into current P (per stream) ---
                if t > 0 and u > 0:
                    nc.tensor.matmul(P[:, 0:HB], Wj[u - 1], spv[:, 0:HB],
                                     start=False, stop=False)
                    nc.tensor.matmul(P[:, HB:FB], Wj[u - 1], spv[:, HB:FB],
                                     start=False, stop=False)

                # --- Act: softplus = Ln(1 + Exp(scale * P)), two streams ---
                sslot = t % 16
                spc = spr[:, sslot * FB:(sslot + 1) * FB]
                sc = float(DEC ** u)
                nc.scalar.activation(Etile[:, 0:HB], P[:, 0:HB], AF.Exp, scale=sc)
                nc.scalar.activation(Etile[:, HB:FB], P[:, HB:FB], AF.Exp, scale=sc)
                nc.scalar.activation(spc[:, 0:HB], Etile[:, 0:HB], AF.Ln, bias=1.0)
                nc.scalar.activation(spc[:, HB:FB], Etile[:, HB:FB], AF.Ln, bias=1.0)
                spv = spc

                # --- PE: last z of the epoch feeds the next epoch's P ---
                if u == K - 1 and t < T - 1:
                    nc.tensor.matmul(Pn[:, 0:HB], Wm1, spc[:, 0:HB],
                                     start=False, stop=False)
                    nc.tensor.matmul(Pn[:, HB:FB], Wm1, spc[:, HB:FB],
                                     start=False, stop=False)

                # --- PE: prefetch dpn matmul for step t+1 (during Ln) ---
                if t + 1 < T:
                    cin = (t + 1) // CH
                    if (t + 1) % CH == 0:
                        dtile = dpnp.tile([128, CH * FB], F16, tag="dpn", name=f"dp{cin}")
                        nc.sync.dma_start(dtile[:], dpn_d[cin])
                        dpn_t[cin] = dtile
                    dsl_n = dpn_t[cin][:, ((t + 1) % CH) * FB:((t + 1) % CH + 1) * FB]
                    un = (t + 1) % K
                    Ptgt = Pn if un == 0 else P
                    nc.tensor.matmul(Ptgt[:], ident[:], dsl_n, start=False, stop=False)

                # --- DVE: S update per stream (writes S ring) ---
                Sprev = Sr[:, ((t - 1) % 16) * FB:((t - 1) % 16 + 1) * FB]
                Scur = Sr[:, (t % 16) * FB:((t % 16) + 1) * FB]
                if t == 0:
                    nc.vector.tensor_scalar(Scur[:, 0:HB], spc[:, 0:HB], 1.0, None, OP.mult)
                    nc.vector.tensor_scalar(Scur[:, HB:FB], spc[:, HB:FB], 1.0, None, OP.mult)
                else:
                    nc.vector.scalar_tensor_tensor(
                        Scur[:, 0:HB], Sprev[:, 0:HB], DEC, spc[:, 0:HB], OP.mult, OP.add)
                    nc.vector.scalar_tensor_tensor(
                        Scur[:, HB:FB], Sprev[:, HB:FB], DEC, spc[:, HB:FB], OP.mult, OP.add)

                # --- deferred bookkeeping for the previous octet ---
                oc = t // OCT
                pos = t % OCT
                od = oc - 1
                if od >= 0:
                    odh = (od % 2) * OCT
                    if pos == 0:
                        spre = Sr[:, ((8 * od - 1) % 16) * FB:(((8 * od - 1) % 16) + 1) * FB]
                        nc.vector.copy_predicated(capS[:], nfo[od % 2][:], spre)
                    elif pos == 1:
                        SrH = Sr[:, odh * FB:(odh + OCT) * FB]
                        nc.vector.tensor_tensor(
                            L1t[:], SrH[:, 0:4 * FB], SrH[:, 4 * FB:8 * FB], OP.max)
                    elif pos == 2:
                        nc.vector.tensor_tensor(
                            L2t[:], L1t[:, 0:2 * FB], L1t[:, 2 * FB:4 * FB], OP.max)
                    elif pos == 3:
                        nc.vector.tensor_tensor(
                            L3t[:], L2t[:, 0:FB], L2t[:, FB:2 * FB], OP.max)
                    elif pos == 4:
                        nc.vector.scalar_tensor_tensor(
                            Fo[:], L3t[:], THR, Fo[:], OP.is_ge, OP.max)
                    elif pos == 5:
                        nc.vector.tensor_scalar(nfo[oc % 2][:], Fo[:], 0.5, None, OP.is_lt)
                        nc.gpsimd.tensor_tensor(cnt[:], cnt[:], nfo[oc % 2][:], OP.add)
                    elif pos == 6:
                        nc.vector.tensor_copy(
                            nfow[oc % 2][:].rearrange("p (a b) -> p a b", a=OCT),
                            nfo[oc % 2][:].unsqueeze(1).broadcast_to([128, OCT, FB]),
                        )
                    elif pos == 7:
                        nc.vector.copy_predicated(
                            capSP[:], nfow[od % 2][:], spr[:, odh * FB:(odh + OCT) * FB])

            # tail: deferred bookkeeping for the last octet (od = 14)
            od = NOCT - 1
            odh = (od % 2) * OCT
            spre = Sr[:, ((8 * od - 1) % 16) * FB:(((8 * od - 1) % 16) + 1) * FB]
            nc.vector.copy_predicated(capS[:], nfo[od % 2][:], spre)
            SrH = Sr[:, odh * FB:(odh + OCT) * FB]
            nc.vector.tensor_tensor(L1t[:], SrH[:, 0:4 * FB], SrH[:, 4 * FB:8 * FB], OP.max)
            nc.vector.tensor_tensor(L2t[:], L1t[:, 0:2 * FB], L1t[:, 2 * FB:4 * FB], OP.max)
            nc.vector.tensor_tensor(L3t[:], L2t[:, 0:FB], L2t[:, FB:2 * FB], OP.max)
            nc.vector.scalar_tensor_tensor(Fo[:], L3t[:], THR, Fo[:], OP.is_ge, OP.max)
            nc.vector.copy_predicated(
                capSP[:], nfow[od % 2][:], spr[:, odh * FB:(odh + OCT) * FB])

            # ================= epilogue =================
            # reconstruct R_j (S values inside the crossing octet)
            Rj = []
            Rprev = capS
            for j in range(OCT):
                R = persist.tile([128, FB], F16, name=f"R{j}")
                nc.vector.scalar_tensor_tensor(
                    R[:], Rprev[:] if j == 0 else Rprev, DEC,
                    capSP[:, j * FB:(j + 1) * FB], OP.mult, OP.add)
                Rj.append(R)
                Rprev = R[:]

            # j* = count of leading below-threshold cummax over R_j
            jstar = work.tile([128, FB], F32, tag="ep2", name="jstar")
            nc.vector.memset(jstar[:], 0.0)
            cm = work.tile([128, FB], F16, tag="ep5", name="cm")
            for j in range(OCT):
                if j == 0:
                    nc.vector.tensor_copy(cm[:], Rj[0][:])
                else:
                    nc.vector.tensor_tensor(cm[:], cm[:], Rj[j][:], OP.max)
                nc.vector.scalar_tensor_tensor(
                    jstar[:], cm[:], THR, jstar[:], OP.is_lt, OP.add)
            cntf = work.tile([128, FB], F32, tag="ep4", name="cntf")
            nc.vector.tensor_copy(cntf[:], cnt[:])
            # idx = 8*(cnt-1) + j*
            idx = work.tile([128, FB], F32, tag="ep", name="idx")
            nc.vector.tensor_scalar(idx[:], cntf[:], 8.0, -8.0, OP.mult, OP.add)
            nc.vector.tensor_tensor(idx[:], idx[:], jstar[:], OP.add)

            Sn = persist.tile([128, FB], F16, name="Sn")
            Sp = persist.tile([128, FB], F16, name="Sp")
            nc.vector.memset(Sn[:], 1.0)
            nc.vector.memset(Sp[:], 0.0)
            for j in range(OCT):
                pj = work.tile([128, FB], I16, tag="pj", name=f"pj{j}")
                nc.vector.tensor_scalar(pj[:], jstar[:], float(j), None, OP.is_equal)
                nc.vector.copy_predicated(Sn[:], pj[:], Rj[j][:])
                nc.vector.copy_predicated(Sp[:], pj[:], capS[:] if j == 0 else Rj[j - 1][:])

            # frac and final time (fp32)
            Sp32 = work.tile([128, FB], F32, tag="f1", name="Sp32")
            nc.vector.tensor_copy(Sp32[:], Sp[:])
            Sn32 = work.tile([128, FB], F32, tag="f2", name="Sn32")
            nc.vector.tensor_copy(Sn32[:], Sn[:])
            den = work.tile([128, FB], F32, tag="f3", name="den")
            nc.vector.tensor_tensor(den[:], Sn32[:], Sp32[:], OP.subtract)
            nc.vector.tensor_scalar(den[:], den[:], EPS5, None, OP.add)
            rec = work.tile([128, FB], F32, tag="f4", name="rec")
            nc.vector.reciprocal(rec[:], den[:])
            num = work.tile([128, FB], F32, tag="f5", name="num")
            nc.vector.tensor_scalar(num[:], Sp32[:], -1.0, THR, OP.mult, OP.add)
            frac = work.tile([128, FB], F32, tag="f6", name="frac")
            nc.vector.tensor_tensor(frac[:], num[:], rec[:], OP.mult)
            mi = work.tile([128, FB], F32, tag="f7", name="mi")
            nc.vector.tensor_scalar(mi[:], idx[:], 0.5, None, OP.is_ge)
            nc.vector.tensor_tensor(frac[:], frac[:], mi[:], OP.mult)
            idx0 = work.tile([128, FB], F32, tag="f8", name="idx0")
            nc.vector.tensor_scalar(idx0[:], idx[:], 1.0, 0.0, OP.subtract, OP.max)
            tval = work.tile([128, FB], F32, tag="f9", name="tval")
            nc.vector.tensor_tensor(tval[:], idx0[:], frac[:], OP.add)
            nc.vector.tensor_scalar(tval[:], tval[:], DT_MS / 1000.0, -1.2, OP.mult, OP.add)
            fnd = work.tile([128, FB], F32, tag="fa", name="fnd")
            nc.vector.tensor_scalar(fnd[:], Fo[:], 0.5, None, OP.is_ge)
            nc.vector.tensor_tensor(tval[:], tval[:], fnd[:], OP.mult)
            nc.vector.tensor_scalar(tval[:], tval[:], 1.2, None, OP.add)
            nc.sync.dma_start(out_d[:], tval[:])
    return nc


def _pin_act_table(nc):
    from concourse import hw_specs as _hs
    import concourse.bacc as _bacc
    full = dict(_hs.get_activation_tables(nc.m.arch))
    keep = "natural_log_exp_and_others"
    patched = {k: (v if k == keep else set()) for k, v in full.items()}
    _bacc.get_activation_tables = lambda arch: patched


last_results = None


def kernel(logits, input_scale, leak, self_excitation, inhibition, noise_std,
           proj_w, proj_b, noise_base):
    logits = np.asarray(logits, dtype=np.float32)
    noise_base = np.asarray(noise_base, dtype=np.float32)
    lk = _softplus(np.asarray(leak))
    se = _softplus(np.asarray(self_excitation))
    inh = float(_softplus(np.asarray(inhibition)))
    ns = float(_softplus(np.asarray(noise_std)))
    alpha = se + inh - lk  # [C]
    w00 = float(np.asarray(proj_w)[0, 0])
    pb0 = float(np.asarray(proj_b)[0])
    iscale = float(np.asarray(input_scale))

    ev = (np.maximum(logits * iscale, 0.0) * w00 + pb0).astype(np.float32)  # [B,C]
    pn = noise_base * np.float32(ns) + ev[None, :, :]                       # [T,B,C]

    p_idx = np.arange(128)
    q_idx = np.arange(128)
    Am = (-inh / 5.0) * (p_idx[:, None] % G == q_idx[None, :] % G).astype(np.float32)
    Am[q_idx, q_idx] += alpha[q_idx // G] / 5.0
    wmats3 = np.zeros((17, 128, 128), dtype=np.float16)
    for j in range(15):
        wmats3[j] = (Am * (DEC ** (-(j + 1)))).astype(np.float16)
    wmats3[15] = Am.astype(np.float16)          # W_m1
    wmats3[16] = (Am * DEC).astype(np.float16)  # Wcar
    wmats = np.ascontiguousarray(wmats3.transpose(1, 0, 2)).reshape(128, 17 * 128)

    # dpn_t = 0.8^-u * pn_t - [u>0] * 0.8^-(u-1) * pn_{t-1},  u = t % K
    u_arr = np.arange(T) % K
    sc = (DEC ** (-u_arr.astype(np.float64))).astype(np.float32)
    dpn = pn * sc[:, None, None]
    dpn[1:][u_arr[1:] > 0] -= pn[:-1][u_arr[1:] > 0] * sc[:-1][u_arr[1:] > 0, None, None]
    dpn = dpn.astype(np.float16)

    nc = bacc.Bacc("TRN2", target_bir_lowering=False, debug=False, num_devices=N_CORES)
    _build(nc)
    _pin_act_table(nc)
    nc.compile()

    in_maps = []
    for c in range(N_CORES):
        s = c * PB
        nz = dpn[:, s:s + PB, :].reshape(T, G, FB, C)
        nz = np.ascontiguousarray(nz.transpose(0, 3, 1, 2)).reshape(T, 128, FB)
        nz = np.ascontiguousarray(
            nz.reshape(T // CH, CH, 128, FB).transpose(0, 2, 1, 3)
        ).reshape(T // CH, 128, CH * FB)
        in_maps.append({"dpn": nz, "wmats": wmats})

    res = bass_utils.run_bass_kernel_spmd(nc, in_maps, core_ids=list(range(N_CORES)))
    global last_results
    last_results = res
    outs = []
    for c in range(N_CORES):
        o = res.results[c]["out"].reshape(C, G, FB)
        outs.append(o.transpose(1, 2, 0).reshape(PB, C))
    return np.concatenate(outs, axis=0)


# revision 20
# speedup vs baseline: 1.4335x; 1.4335x over previous
import os
import sys

for _p in ("/opt/trn_rl_repo", "/root/.axon_site/_ro/trn_rl_repo"):
    if os.path.isdir(_p) and _p not in sys.path:
        sys.path.insert(0, _p)

import numpy as np

import concourse.bass as bass
import concourse.mybir as mybir
from concourse.tile import TileContext
from concourse import bass_utils
from concourse import bacc

F16 = mybir.dt.float16
F32 = mybir.dt.float32
I16 = mybir.dt.int16
I32 = mybir.dt.int32
AF = mybir.ActivationFunctionType
OP = mybir.AluOpType

N_CORES = 8
BATCH = 65536
C = 4                  # classes
T = 120                # time steps
PB = BATCH // N_CORES  # batch per core = 8192
G = 32                 # groups (partitions = 4 classes x 32 groups)
FB = PB // G           # free-dim batch per partition = 256
HB = FB // 2           # half-width per stream = 128
K = 16                 # epoch length (0.8^-u stays fp16-safe)
OCT = 8                # octet length for bookkeeping
NOCT = T // OCT        # 15 octets
CH = 8                 # dpn steps per DMA chunk
DT_MS = 10.0
THR = 2.5              # threshold in S=5*acc units
EPS5 = 5e-9
DEC = 0.8


def _softplus(x):
    return np.logaddexp(0.0, x.astype(np.float64)).astype(np.float32)


def _build(nc):
    dpn_d = nc.dram_tensor("dpn", [T // CH, 128, CH * FB], F16, kind="ExternalInput")
    w_d = nc.dram_tensor("wmats", [128, 17 * 128], F16, kind="ExternalInput")
    out_d = nc.dram_tensor("out", [128, FB], F32, kind="ExternalOutput")

    with TileContext(nc) as tc:
        with (
            tc.tile_pool(name="persist", bufs=1) as persist,
            tc.tile_pool(name="dpnp", bufs=3) as dpnp,
            tc.tile_pool(name="work", bufs=2) as work,
            tc.tile_pool(name="psum", bufs=1, space="PSUM") as psump,
        ):
            # --- stationary weights: W_j (j=0..14), W_m1=A, Wcar=0.8*A ---
            wall = persist.tile([128, 17 * 128], F16, name="wall")
            nc.sync.dma_start(wall[:], w_d[:])
            Wj = [wall[:, j * 128:(j + 1) * 128] for j in range(15)]
            Wm1 = wall[:, 15 * 128:16 * 128]
            Wcar = wall[:, 16 * 128:17 * 128]
            ident = persist.tile([128, 128], F16, name="ident")
            rowi = persist.tile([128, 128], I32, name="rowi")
            nc.gpsimd.iota(rowi[:], [[1, 128]], base=0, channel_multiplier=0)
            coli = persist.tile([128, 1], I32, name="coli")
            nc.gpsimd.iota(coli[:], [[0, 1]], base=0, channel_multiplier=1)
            row = persist.tile([128, 128], F32, name="row")
            nc.vector.tensor_copy(row[:], rowi[:])
            col = persist.tile([128, 1], F32, name="col")
            nc.vector.tensor_copy(col[:], coli[:])
            nc.vector.tensor_scalar(ident[:], row[:], col[:], None, OP.is_equal)

            # --- state rings (slot t at ring position t%16; each slot holds
            # stream A in cols [0:HB) and stream B in cols [HB:FB)) ---
            spr = persist.tile([128, 16 * FB], F16, name="spr")   # sp ring
            Sr = persist.tile([128, 16 * FB], F16, name="Sr")     # S ring
            nc.vector.memset(Sr[:, 15 * FB:16 * FB], 0.0)         # S_{-1} = 0
            Fo = persist.tile([128, FB], F16, name="Fo")
            nc.vector.memset(Fo[:], 0.0)
            cnt = persist.tile([128, FB], I32, name="cnt")
            nc.vector.memset(cnt[:], 1)                           # octet-0 latch
            capS = persist.tile([128, FB], F16, name="capS")
            capSP = persist.tile([128, OCT * FB], F16, name="capSP")
            nc.vector.memset(capS[:], 0.0)
            nc.vector.memset(capSP[:], 0.0)
            nfo = [persist.tile([128, FB], I32, name=f"nfo{i}") for i in range(2)]
            nfow = [persist.tile([128, OCT * FB], I16, name=f"nfow{i}") for i in range(2)]
            nc.vector.memset(nfo[0][:], 1)
            nc.vector.memset(nfow[0][:], 1)
            L1t = persist.tile([128, 4 * FB], F16, name="L1t")
            L2t = persist.tile([128, 2 * FB], F16, name="L2t")
            L3t = persist.tile([128, FB], F16, name="L3t")

            # --- PSUM accumulators ---
            P0 = psump.tile([128, FB], F32, name="P0")
            P1 = psump.tile([128, FB], F32, name="P1")
            E0 = psump.tile([128, FB], F32, name="E0")
            E1 = psump.tile([128, FB], F32, name="E1")
            Ps = [P0, P1]

            dpn_t = [None] * (T // CH)
            dt0 = dpnp.tile([128, CH * FB], F16, tag="dpn", name="dp0")
            nc.sync.dma_start(dt0[:], dpn_d[0])
            dpn_t[0] = dt0
            nc.tensor.matmul(P0[:], ident[:], dt0[:, 0:FB], start=True, stop=False)
            spv = None  # sp ring slot AP of previous step

            for t in range(T):
                u = t % K
                e = t // K
                P = Ps[e % 2]
                Pn = Ps[(e + 1) % 2]
                Etile = E0 if (t % 2 == 0) else E1

                # --- PE: carry for next epoch (start=True on Pn) first ---
                if u == K - 1 and t < T - 1:
                    sprev2 = Sr[:, ((t - 1) % 16) * FB:((t - 1) % 16 + 1) * FB]
                    nc.tensor.matmul(Pn[:], Wcar, sprev2, start=True, stop=False)

                # --- PE: z of previous sp into current P ---
                if t > 0 and u > 0:
                    nc.tensor.matmul(P[:], Wj[u - 1], spv, start=False, stop=False)

                # --- Act: softplus = Ln(1 + Exp(scale * P)), two streams ---
                sslot = t % 16
                spc = spr[:, sslot * FB:(sslot + 1) * FB]
                nc.scalar.activation(Etile[:], P[:], AF.Exp, scale=float(DEC ** u))
                nc.scalar.activation(spc, Etile[:], AF.Ln, bias=1.0)
                spv = spc

                # --- PE: last z of the epoch feeds the next epoch's P ---
                if u == K - 1 and t < T - 1:
                    nc.tensor.matmul(Pn[:], Wm1, spc, start=False, stop=False)

                # --- PE: prefetch dpn matmul for step t+1 (during Ln) ---
                if t + 1 < T:
                    cin = (t + 1) // CH
                    if (t + 1) % CH == 0:
                        dtile = dpnp.tile([128, CH * FB], F16, tag="dpn", name=f"dp{cin}")
                        nc.sync.dma_start(dtile[:], dpn_d[cin])
                        dpn_t[cin] = dtile
                    dsl_n = dpn_t[cin][:, ((t + 1) % CH) * FB:((t + 1) % CH + 1) * FB]
                    un = (t + 1) % K
                    Ptgt = Pn if un == 0 else P
                    nc.tensor.matmul(Ptgt[:], ident[:], dsl_n, start=False, stop=False)

                # --- DVE: S update per stream (writes S ring) ---
                Sprev = Sr[:, ((t - 1) % 16) * FB:((t - 1) % 16 + 1) * FB]
                Scur = Sr[:, (t % 16) * FB:((t % 16) + 1) * FB]
                if t == 0:
                    nc.vector.tensor_scalar(Scur, spc, 1.0, None, OP.mult)
                else:
                    nc.vector.scalar_tensor_tensor(Scur, Sprev, DEC, spc, OP.mult, OP.add)

                # --- deferred bookkeeping for the previous octet ---
                oc = t // OCT
                pos = t % OCT
                od = oc - 1
                if od >= 0:
                    odh = (od % 2) * OCT
                    if pos == 0:
                        spre = Sr[:, ((8 * od - 1) % 16) * FB:(((8 * od - 1) % 16) + 1) * FB]
                        nc.vector.copy_predicated(capS[:], nfo[od % 2][:], spre)
                    elif pos == 1:
                        SrH = Sr[:, odh * FB:(odh + OCT) * FB]
                        nc.vector.tensor_tensor(
                            L1t[:], SrH[:, 0:4 * FB], SrH[:, 4 * FB:8 * FB], OP.max)
                    elif pos == 2:
                        nc.vector.tensor_tensor(
                            L2t[:], L1t[:, 0:2 * FB], L1t[:, 2 * FB:4 * FB], OP.max)
                    elif pos == 3:
                        nc.vector.tensor_tensor(
                            L3t[:], L2t[:, 0:FB], L2t[:, FB:2 * FB], OP.max)
                    elif pos == 4:
                        nc.vector.scalar_tensor_tensor(
                            Fo[:], L3t[:], THR, Fo[:], OP.is_ge, OP.max)
                    elif pos == 5:
                        nc.vector.tensor_scalar(nfo[oc % 2][:], Fo[:], 0.5, None, OP.is_lt)
                        nc.gpsimd.tensor_tensor(cnt[:], cnt[:], nfo[oc % 2][:], OP.add)
                    elif pos == 6:
                        nc.vector.tensor_copy(
                            nfow[oc % 2][:].rearrange("p (a b) -> p a b", a=OCT),
                            nfo[oc % 2][:].unsqueeze(1).broadcast_to([128, OCT, FB]),
                        )
                    elif pos == 7:
                        nc.vector.copy_predicated(
                            capSP[:], nfow[od % 2][:], spr[:, odh * FB:(odh + OCT) * FB])

            # tail: deferred bookkeeping for the last octet (od = 14)
            od = NOCT - 1
            odh = (od % 2) * OCT
            spre = Sr[:, ((8 * od - 1) % 16) * FB:(((8 * od - 1) % 16) + 1) * FB]
            nc.vector.copy_predicated(capS[:], nfo[od % 2][:], spre)
            SrH = Sr[:, odh * FB:(odh + OCT) * FB]
            nc.vector.tensor_tensor(L1t[:], SrH[:, 0:4 * FB], SrH[:, 4 * FB:8 * FB], OP.max)
            nc.vector.tensor_tensor(L2t[:], L1t[:, 0:2 * FB], L1t[:, 2 * FB:4 * FB], OP.max)
            nc.vector.tensor_tensor(L3t[:], L2t[:, 0:FB], L2t[:, FB:2 * FB], OP.max)
            nc.vector.scalar_tensor_tensor(Fo[:], L3t[:], THR, Fo[:], OP.is_ge, OP.max)
            nc.vector.copy_predicated(
                capSP[:], nfow[od % 2][:], spr[:, odh * FB:(odh + OCT) * FB])

            # ================= epilogue =================
            # reconstruct R_j (S values inside the crossing octet)
            Rj = []
            Rprev = capS
            for j in range(OCT):
                R = persist.tile([128, FB], F16, name=f"R{j}")
                nc.vector.scalar_tensor_tensor(
                    R[:], Rprev[:] if j == 0 else Rprev, DEC,
                    capSP[:, j * FB:(j + 1) * FB], OP.mult, OP.add)
                Rj.append(R)
                Rprev = R[:]

            # j* = count of leading below-threshold cummax over R_j
            jstar = work.tile([128, FB], F32, tag="ep2", name="jstar")
            nc.vector.memset(jstar[:], 0.0)
            cm = work.tile([128, FB], F16, tag="ep5", name="cm")
            for j in range(OCT):
                if j == 0:
                    nc.vector.tensor_copy(cm[:], Rj[0][:])
                else:
                    nc.vector.tensor_tensor(cm[:], cm[:], Rj[j][:], OP.max)
                nc.vector.scalar_tensor_tensor(
                    jstar[:], cm[:], THR, jstar[:], OP.is_lt, OP.add)
            cntf = work.tile([128, FB], F32, tag="ep4", name="cntf")
            nc.vector.tensor_copy(cntf[:], cnt[:])
            # idx = 8*(cnt-1) + j*
            idx = work.tile([128, FB], F32, tag="ep", name="idx")
            nc.vector.tensor_scalar(idx[:], cntf[:], 8.0, -8.0, OP.mult, OP.add)
            nc.vector.tensor_tensor(idx[:], idx[:], jstar[:], OP.add)

            Sn = persist.tile([128, FB], F16, name="Sn")
            Sp = persist.tile([128, FB], F16, name="Sp")
            nc.vector.memset(Sn[:], 1.0)
            nc.vector.memset(Sp[:], 0.0)
            for j in range(OCT):
                pj = work.tile([128, FB], I16, tag="pj", name=f"pj{j}")
                nc.vector.tensor_scalar(pj[:], jstar[:], float(j), None, OP.is_equal)
                nc.vector.copy_predicated(Sn[:], pj[:], Rj[j][:])
                nc.vector.copy_predicated(Sp[:], pj[:], capS[:] if j == 0 else Rj[j - 1][:])

            # frac and final time (fp32)
            Sp32 = work.tile([128, FB], F32, tag="f1", name="Sp32")
            nc.vector.tensor_copy(Sp32[:], Sp[:])
            Sn32 = work.tile([128, FB], F32, tag="f2", name="Sn32")
            nc.vector.tensor_copy(Sn32[:], Sn[:])
            den = work.tile([128, FB], F32, tag="f3", name="den")
            nc.vector.tensor_tensor(den[:], Sn32[:], Sp32[:], OP.subtract)
            nc.vector.tensor_scalar(den[:], den[:], EPS5, None, OP.add)
            rec = work.tile([128, FB], F32, tag="f4", name="rec")
            nc.vector.reciprocal(rec[:], den[:])
            num = work.tile([128, FB], F32, tag="f5", name="num")
            nc.vector.tensor_scalar(num[:], Sp32[:], -1.0, THR, OP.mult, OP.add)
            frac = work.tile([128, FB], F32, tag="f6", name="frac")
            nc.vector.tensor_tensor(frac[:], num[:], rec[:], OP.mult)
            mi = work.tile([128, FB], F32, tag="f7", name="mi")
            nc.vector.tensor_scalar(mi[:], idx[:], 0.5, None, OP.is_ge)
            nc.vector.tensor_tensor(frac[:], frac[:], mi[:], OP.mult)
            idx0 = work.tile([128, FB], F32, tag="f8", name="idx0")
            nc.vector.tensor_scalar(idx0[:], idx[:], 1.0, 0.0, OP.subtract, OP.max)
            tval = work.tile([128, FB], F32, tag="f9", name="tval")
            nc.vector.tensor_tensor(tval[:], idx0[:], frac[:], OP.add)
            nc.vector.tensor_scalar(tval[:], tval[:], DT_MS / 1000.0, -1.2, OP.mult, OP.add)
            fnd = work.tile([128, FB], F32, tag="fa", name="fnd")
            nc.vector.tensor_scalar(fnd[:], Fo[:], 0.5, None, OP.is_ge)
            nc.vector.tensor_tensor(tval[:], tval[:], fnd[:], OP.mult)
            nc.vector.tensor_scalar(tval[:], tval[:], 1.2, None, OP.add)
            nc.sync.dma_start(out_d[:], tval[:])
    return nc


def _pin_act_table(nc):
    from concourse import hw_specs as _hs
    import concourse.bacc as _bacc
    full = dict(_hs.get_activation_tables(nc.m.arch))
    keep = "natural_log_exp_and_others"
    patched = {k: (v if k == keep else set()) for k, v in full.items()}
    _bacc.get_activation_tables = lambda arch: patched


last_results = None


def kernel(logits, input_scale, leak, self_excitation, inhibition, noise_std,
           proj_w, proj_b, noise_base):
    logits = np.asarray(logits, dtype=np.float32)
    noise_base = np.asarray(noise_base, dtype=np.float32)
    lk = _softplus(np.asarray(leak))
    se = _softplus(np.asarray(self_excitation))
    inh = float(_softplus(np.asarray(inhibition)))
    ns = float(_softplus(np.asarray(noise_std)))
    alpha = se + inh - lk  # [C]
    w00 = float(np.asarray(proj_w)[0, 0])
    pb0 = float(np.asarray(proj_b)[0])
    iscale = float(np.asarray(input_scale))

    ev = (np.maximum(logits * iscale, 0.0) * w00 + pb0).astype(np.float32)  # [B,C]
    pn = noise_base * np.float32(ns) + ev[None, :, :]                       # [T,B,C]

    p_idx = np.arange(128)
    q_idx = np.arange(128)
    Am = (-inh / 5.0) * (p_idx[:, None] % G == q_idx[None, :] % G).astype(np.float32)
    Am[q_idx, q_idx] += alpha[q_idx // G] / 5.0
    wmats3 = np.zeros((17, 128, 128), dtype=np.float16)
    for j in range(15):
        wmats3[j] = (Am * (DEC ** (-(j + 1)))).astype(np.float16)
    wmats3[15] = Am.astype(np.float16)          # W_m1
    wmats3[16] = (Am * DEC).astype(np.float16)  # Wcar
    wmats = np.ascontiguousarray(wmats3.transpose(1, 0, 2)).reshape(128, 17 * 128)

    # dpn_t = 0.8^-u * pn_t - [u>0] * 0.8^-(u-1) * pn_{t-1},  u = t % K
    u_arr = np.arange(T) % K
    sc = (DEC ** (-u_arr.astype(np.float64))).astype(np.float32)
    dpn = pn * sc[:, None, None]
    dpn[1:][u_arr[1:] > 0] -= pn[:-1][u_arr[1:] > 0] * sc[:-1][u_arr[1:] > 0, None, None]
    dpn = dpn.astype(np.float16)

    nc = bacc.Bacc("TRN2", target_bir_lowering=False, debug=False, num_devices=N_CORES)
    _build(nc)
    _pin_act_table(nc)
    nc.compile()

    in_maps = []
    for c in range(N_CORES):
        s = c * PB
        nz = dpn[:, s:s + PB, :].reshape(T, G, FB, C)
        nz = np.ascontiguousarray(nz.transpose(0, 3, 1, 2)).reshape(T, 128, FB)
        nz = np.ascontiguousarray(
            nz.reshape(T // CH, CH, 128, FB).transpose(0, 2, 1, 3)
        ).reshape(T // CH, 128, CH * FB)
        in_maps.append({"dpn": nz, "wmats": wmats})

    res = bass_utils.run_bass_kernel_spmd(nc, in_maps, core_ids=list(range(N_CORES)))
    global last_results
    last_results = res
    outs = []
    for c in range(N_CORES):
        o = res.results[c]["out"].reshape(C, G, FB)
        outs.append(o.transpose(1, 2, 0).reshape(PB, C))
    return np.concatenate(outs, axis=0)
